# revision 18
# baseline (speedup 1.0000x reference)
"""Trainium2 Bass kernel for nn_HadamardTransform: Y = X @ H4096_normalized.

Algorithm: H4096 (Sylvester, normalized) factors exactly as the Kronecker
product H32n (x) H128n.  Each row x of X, reshaped row-major to R[32, 128],
transforms as  Y_mat = G @ R @ H128u  with G = 2^-6 * H32u (all of the
2^-6 normalization folded into the 32-side so H128u stays exactly +-1).

On-chip scheme per 128x128 tile T (4 consecutive rows, SBUF partition
p = 32*rr + i, free = j, where column c = 128*i + j):
  MM-A: psumA = T.T @ W1      (W1 = I4 (x) G, block-diagonal 128x128)
        -> psumA[j, (rr,i')] : the i-transform, emerging j-on-partitions
  MM-B: psumB = psumA.T @ H128u
        -> psumB[(rr,i'), j'] : the j-transform, natural output layout
No transposes are needed anywhere; the fixed matrices W1/H128u are the
moving operands, the per-tile data is the stationary operand.

Everything on device is fp16: X is cast host-side (rel err ~5e-4, vs the
2e-2 gate), halving DMA traffic (the roofline here), and fp16 moving
operands stream at 1 cycle/column through the PE vs 4 for fp32.  PSUM
accumulation stays fp32; the PSUM->SBUF evacuation casts to fp16.

Engine assignment: SP issues loads, Pool (gpsimd) issues stores, DVE
evacuates psumA, ACT evacuates psumB -- four independent queues so the
DMA rings and the PE never wait on a shared sequencer.

Sharding: X's 8192 rows split into 8 contiguous shards of 1024 rows, one
per NeuronCore (pure data parallelism, no collectives).
"""

import sys

import numpy as np

try:
    import concourse.bass as bass
except ImportError:  # repo not on sys.path in a fresh grading dir
    sys.path.insert(0, "/opt/trn_rl_repo")
    import concourse.bass as bass

import concourse.mybir as mybir
import concourse.tile as tile
from concourse import bacc
from concourse.bass_utils import run_bass_kernel_spmd

N_CORES = 8
ROWS = 8192
N = 4096
ROWS_PER_CORE = ROWS // N_CORES  # 1024
ROWS_PER_GROUP = 64              # rows moved per DMA (512 KiB fp16)
A_PER_GROUP = ROWS_PER_GROUP // 4  # 16 column-chunks of 128 per group
HALVES = A_PER_GROUP // 4          # 4 psum-tile passes per group
GROUPS = ROWS_PER_CORE // ROWS_PER_GROUP  # 16
F16 = mybir.dt.float16
F32 = mybir.dt.float32


def _hadamard_u(n: int) -> np.ndarray:
    """Unnormalized Sylvester Hadamard matrix (+-1 entries)."""
    H = np.array([[1.0]], dtype=np.float64)
    while H.shape[0] < n:
        H = np.block([[H, H], [H, -H]])
    return H


def _constants() -> tuple[np.ndarray, np.ndarray]:
    G = (2.0 ** -6) * _hadamard_u(32)          # fold full 2^-6 norm here
    W1 = np.kron(np.eye(4), G).astype(np.float16)   # [128,128] block-diag
    HJ = _hadamard_u(128).astype(np.float16)        # [128,128] exact +-1
    return W1, HJ


def _build_bass(loop_reps: int | None = None, mode: str = "full"):
    """loop_reps: if set, wrap the whole body in a HW For_i loop that
    repeats it loop_reps times (timing harness only — result unchanged
    since the same X is re-read).
    mode: "full" (real kernel), "dma" (loads+stores only, no compute),
    "compute" (matmuls+copies on resident tiles, no X/Y DMA) — the
    latter two are bottleneck-isolation probes for the timing harness."""
    nc = bacc.Bacc("TRN2", target_bir_lowering=False, debug=False)

    X = nc.dram_tensor("X", [ROWS_PER_CORE, N], F16, kind="ExternalInput")
    W1 = nc.dram_tensor("W1", [128, 128], F16, kind="ExternalInput")
    HJ = nc.dram_tensor("HJ", [128, 128], F16, kind="ExternalInput")
    Y = nc.dram_tensor("Y", [ROWS_PER_CORE, N], F16, kind="ExternalOutput")

    # row r = 64*g + 4*a + b ; column c = 128*i + j
    # SBUF group tile: partition p = 32*b + i, free f = 128*a + j
    X_re = X[:].rearrange(
        "(g a b) (i j) -> g b i a j", a=A_PER_GROUP, b=4, i=32, j=128
    )
    Y_re = Y[:].rearrange(
        "(g a b) (i j) -> g b i a j", a=A_PER_GROUP, b=4, i=32, j=128
    )
    # store-side view split into two a-halves per group, so each half of a
    # group's output can stream out as soon as its ACT evacuations finish
    Y_re_half = Y[:].rearrange(
        "(g ah a b) (i j) -> g ah b i a j",
        ah=2, a=A_PER_GROUP // 2, b=4, i=32, j=128,
    )
    FREE = A_PER_GROUP * 128  # free size of a group tile

    if mode == "dma512":
        # DMA probe with 512-byte contiguous runs (partition = 8 rows x 16
        # column superblocks of 256): same bytes, H16(x)H256-style layout.
        X_re = X[:].rearrange(
            "(g a b) (i j) -> g b i a j", a=A_PER_GROUP // 2, b=8, i=16, j=256
        )
        Y_re = Y[:].rearrange(
            "(g a b) (i j) -> g b i a j", a=A_PER_GROUP // 2, b=8, i=16, j=256
        )
    if mode == "dmanat":
        # DMA probe, fully contiguous: 128 consecutive rows per tile, 8 KiB
        # per partition line.
        X_re = X[:].rearrange("(g p) c -> g p c", p=128)
        Y_re = Y[:].rearrange("(g p) c -> g p c", p=128)

    with tile.TileContext(nc) as tc:
        with (
            tc.tile_pool(name="consts", bufs=1) as cpool,
            tc.tile_pool(name="xin", bufs=5) as xpool,
            tc.tile_pool(name="yout", bufs=4) as ypool,
            tc.tile_pool(name="mid", bufs=6) as spool,
            tc.tile_pool(name="psA", bufs=3, space="PSUM") as psA,
            tc.tile_pool(name="psB", bufs=3, space="PSUM") as psB,
        ):
            w1 = cpool.tile([128, 128], F16)
            nc.sync.dma_start(out=w1[:], in_=W1[:])
            hj = cpool.tile([128, 128], F16)
            nc.sync.dma_start(out=hj[:], in_=HJ[:])
            xconst = None
            if mode == "compute":
                # resident input tile so the PE pipeline runs without any
                # per-group DMA (bottleneck-isolation probe)
                xconst = cpool.tile([128, FREE], F16)
                nc.sync.dma_start(
                    out=xconst[:].rearrange("p (a j) -> p a j",
                                            a=A_PER_GROUP, j=128),
                    in_=X_re[0],
                )

            def flush_b(state):
                """Emit the B-stage (MM-B x4 + ACT copy + maybe store)
                for a previously A-staged half-group."""
                if state is None:
                    return
                sa, yw_3d_, yw_, h_, g_ = state
                pb = psB.tile([128, 512], F32)
                for q in range(4):
                    nc.tensor.matmul(
                        pb[:, q * 128:(q + 1) * 128],
                        lhsT=sa[:, q * 128:(q + 1) * 128],
                        rhs=hj[:],
                        start=True,
                        stop=True,
                    )
                nc.scalar.copy(
                    out=yw_[:, h_ * 512:(h_ + 1) * 512], in_=pb[:]
                )
                if h_ % (HALVES // 2) == HALVES // 2 - 1 and mode != "compute":
                    # stores ride the ACT HWDGE ring; loads own the SP
                    # ring (a shared FIFO ring head-of-line-blocks loads
                    # behind stores that wait on compute).  gpsimd would
                    # route through SWDGE (per-descriptor Q7 software
                    # generation) — far slower for this 2048-desc pattern.
                    # Each a-half of the group streams out as soon as its
                    # two ACT evacuations land.
                    ah = h_ // (HALVES // 2)
                    half = yw_[:, ah * (FREE // 2):(ah + 1) * (FREE // 2)]
                    nc.scalar.dma_start(
                        out=Y_re_half[g_, ah],
                        in_=half.rearrange("p (a j) -> p a j",
                                           a=A_PER_GROUP // 2, j=128),
                    )

            def emit_body():
              # 1-stage software pipeline: each half-group's MM-B block is
              # emitted after the NEXT half-group's MM-A block, so the PE
              # FIFO never stalls on the DVE PSUM->SBUF copy in between.
              prev = None
              for g in range(GROUPS):
                if mode == "dmanat":
                    if g >= ROWS_PER_CORE // 128:
                        continue
                    xn = xpool.tile([128, 4096], F16)
                    nc.sync.dma_start(out=xn[:], in_=X_re[g])
                    nc.scalar.dma_start(out=Y_re[g], in_=xn[:])
                    continue
                if mode == "compute":
                    xw = xconst
                else:
                    xw = xpool.tile([128, FREE], F16)
                # SBUF partition dim must stay a single dim0; DRAM side
                # enumerates (b, i, a, j) which matches (p, a, j) order.
                xw_3d = xw[:].rearrange("p (a j) -> p a j", a=A_PER_GROUP, j=128)
                if mode == "dma512":
                    xw_p = xw[:].rearrange("p (a j) -> p a j",
                                           a=A_PER_GROUP // 2, j=256)
                    nc.sync.dma_start(out=xw_p, in_=X_re[g])
                    nc.scalar.dma_start(out=Y_re[g], in_=xw_p)
                    continue
                if mode == "dma3":
                    le = nc.sync if g % 2 == 0 else nc.scalar
                    se = nc.scalar if g % 2 == 0 else nc.sync
                    le.dma_start(out=xw_3d, in_=X_re[g])
                    se.dma_start(out=Y_re[g], in_=xw_3d)
                    continue
                if mode == "dma2":
                    # ring-split probe: loads alternate SP/ACT HWDGE rings,
                    # stores ride the gpsimd SWDGE ring.
                    eng = nc.sync if g % 2 == 0 else nc.scalar
                    eng.dma_start(out=xw_3d, in_=X_re[g])
                elif mode != "compute":
                    nc.sync.dma_start(out=xw_3d, in_=X_re[g])
                yw = ypool.tile([128, FREE], F16)
                yw_3d = yw[:].rearrange("p (a j) -> p a j", a=A_PER_GROUP, j=128)
                if mode == "dma":
                    # store the loaded tile back out; no compute involved.
                    nc.scalar.dma_start(out=Y_re[g], in_=xw_3d)
                    continue
                if mode == "dmaload":
                    # loads only — probes read bandwidth without store mixing.
                    # (xw unused downstream; a second dummy read keeps the
                    # tile "accessed" so the pool bookkeeping stays happy.)
                    nc.vector.tensor_copy(out=xw[:, 0:8], in_=xw[:, 0:8])
                    continue
                for h in range(HALVES):
                    pa = psA.tile([128, 512], F32)
                    for q in range(4):
                        rg = 4 * h + q
                        nc.tensor.matmul(
                            pa[:, q * 128:(q + 1) * 128],
                            lhsT=xw[:, rg * 128:(rg + 1) * 128],
                            rhs=w1[:],
                            start=True,
                            stop=True,
                        )
                    flush_b(prev)
                    sa = spool.tile([128, 512], F16)
                    nc.vector.tensor_copy(out=sa[:], in_=pa[:])
                    prev = (sa, yw_3d, yw, h, g)
              flush_b(prev)

            if loop_reps is None:
                emit_body()
            else:
                with tc.For_i(0, loop_reps, 1):
                    emit_body()

    nc.compile()
    return nc


_NC = None


def _get_nc():
    global _NC
    if _NC is None:
        _NC = _build_bass()
    return _NC


def make_in_maps(X: np.ndarray) -> list[dict]:
    """Shard X row-wise into 8 fp16 per-core input maps."""
    X = np.asarray(X, dtype=np.float32)
    assert X.shape == (ROWS, N), X.shape
    X16 = X.astype(np.float16)
    W1, HJ = _constants()
    return [
        {
            "X": X16[c * ROWS_PER_CORE:(c + 1) * ROWS_PER_CORE],
            "W1": W1,
            "HJ": HJ,
        }
        for c in range(N_CORES)
    ]


def run(X: np.ndarray, trace: bool = False):
    """Run the SPMD kernel on 8 cores; returns (Y, BassKernelResults)."""
    in_maps = make_in_maps(X)
    nc = _get_nc()
    res = run_bass_kernel_spmd(
        nc, in_maps, list(range(N_CORES)), trace=trace
    )
    Y = np.concatenate(
        [res.results[c]["Y"] for c in range(N_CORES)], axis=0
    ).astype(np.float32)
    return Y, res


def kernel(X, H=None, **_unused) -> np.ndarray:
    """Full-input entry point: X (8192, 4096) f32, H ignored (H is the
    deterministic normalized Hadamard matrix, synthesized on device)."""
    Y, _ = run(X, trace=False)
    return Y


# revision 23
# speedup vs baseline: 1.4561x; 1.4561x over previous
"""Trainium2 Bass kernel for nn_HadamardTransform: Y = X @ H4096_normalized.

Algorithm: H4096 (Sylvester, normalized) factors exactly as the Kronecker
product H16n (x) H256n.  Each row x of X, reshaped row-major to R[16, 256],
transforms as  Y_mat = G16 @ R @ H256u  with G16 = 2^-6 * H16u (all of the
2^-6 normalization folded into the 16-side so H256u stays exactly +-1).

On-chip scheme per 8-row slice (partition p = 16*b + ip, b in [8] rows,
ip in [16] column superblocks of 256; free = (a, jc in [256])):
  MM-A (x2, one per 128-half jh of jc):
      psumA[jh-half] = T_jh.T @ W1      (W1 = I8 (x) G16, block-diagonal)
      -> psumA[jc, (b,i')] : the 16-transform, emerging jc-on-partitions
      (the two jc halves land in different psum tiles' partition meaning,
       packed as columns [0:128]/[128:256] of one psum bank)
  MM-B (x4, accumulating pairs): psumB[:, ch] = sum_h sah.T @ H256[h, ch]
      -> psumB[(b,i'), jc'] : the 256-transform, natural output layout
No transposes are needed anywhere; the fixed matrices are the moving
operands, the per-slice data is the stationary operand.

Why 16x256 rather than 32x128: the DMA descriptor economy.  Both layouts
move the same bytes, but contiguous runs here are 512 B (256 fp16) vs
256 B, halving descriptor count; measured on HW the 8-core DMA floor
drops from ~67 us to ~60.5 us (payload-bound at ~278 GB/s/core), at the
cost of a third PE pass (the 256-contraction needs two accumulating
matmuls).  PE and DMA both land near 60 us and overlap.

Everything on device is fp16: X is cast host-side (rel err ~5e-4 vs the
2e-2 gate), halving DMA traffic (the roofline here); fp16 moving
operands stream at 1 cycle/column through the PE vs 4 for fp32.  PSUM
accumulation stays fp32; the PSUM->SBUF evacuations cast to fp16 (DVE
for stage A, ACT for stage B).  Loads ride the SP HWDGE ring, stores the
ACT HWDGE ring.

Sharding: X's 8192 rows split into 8 contiguous shards of 1024 rows, one
per NeuronCore (pure data parallelism, no collectives).
"""

import sys

import numpy as np

try:
    import concourse.bass as bass
except ImportError:  # repo not on sys.path in a fresh grading dir
    sys.path.insert(0, "/opt/trn_rl_repo")
    import concourse.bass as bass

import concourse.mybir as mybir
import concourse.tile as tile
from concourse import bacc
from concourse.bass_utils import run_bass_kernel_spmd

N_CORES = 8
ROWS = 8192
N = 4096
ROWS_PER_CORE = ROWS // N_CORES  # 1024
ROWS_PER_GROUP = 64               # rows moved per DMA (512 KiB fp16)
A_PER_GROUP = ROWS_PER_GROUP // 8  # 8-row slices per group
GROUPS = ROWS_PER_CORE // ROWS_PER_GROUP  # 16
F16 = mybir.dt.float16
F32 = mybir.dt.float32


def _hadamard_u(n: int) -> np.ndarray:
    """Unnormalized Sylvester Hadamard matrix (+-1 entries)."""
    H = np.array([[1.0]], dtype=np.float64)
    while H.shape[0] < n:
        H = np.block([[H, H], [H, -H]])
    return H


def _constants() -> tuple[np.ndarray, np.ndarray]:
    G16 = (2.0 ** -6) * _hadamard_u(16)         # fold full 2^-6 norm here
    W1 = np.kron(np.eye(8), G16).astype(np.float16)  # [128,128] block-diag
    HJ2 = _hadamard_u(256).astype(np.float16)        # [256,256] exact +-1
    return W1, HJ2


def _build_bass(loop_reps: int | None = None, mode: str = "full"):
    """loop_reps: if set, wrap the whole body in a HW For_i loop that
    repeats it loop_reps times (timing harness only — result unchanged
    since the same X is re-read).
    mode: "full" (real kernel), "dma" (loads+stores only, no compute),
    "compute" (matmuls+copies on resident tiles, no X/Y DMA)."""
    nc = bacc.Bacc("TRN2", target_bir_lowering=False, debug=False)

    X = nc.dram_tensor("X", [ROWS_PER_CORE, N], F16, kind="ExternalInput")
    W1 = nc.dram_tensor("W1", [128, 128], F16, kind="ExternalInput")
    HJ2 = nc.dram_tensor("HJ2", [256, 256], F16, kind="ExternalInput")
    Y = nc.dram_tensor("Y", [ROWS_PER_CORE, N], F16, kind="ExternalOutput")

    # row r = 64*g + 8*a + b ; column c = 256*ip + jc
    # SBUF group tile: partition p = 16*b + ip, free f = 256*a + jc
    X_re = X[:].rearrange(
        "(g a b) (i j) -> g b i a j", a=A_PER_GROUP, b=8, i=16, j=256
    )
    Y_re = Y[:].rearrange(
        "(g a b) (i j) -> g b i a j", a=A_PER_GROUP, b=8, i=16, j=256
    )
    FREE = A_PER_GROUP * 256  # free size of a group tile

    with tile.TileContext(nc) as tc:
        with (
            tc.tile_pool(name="consts", bufs=1) as cpool,
            tc.tile_pool(name="xin", bufs=5) as xpool,
            tc.tile_pool(name="yout", bufs=4) as ypool,
            tc.tile_pool(name="mid", bufs=6) as spool,
            tc.tile_pool(name="psA", bufs=3, space="PSUM") as psA,
            tc.tile_pool(name="psB", bufs=3, space="PSUM") as psB,
        ):
            w1 = cpool.tile([128, 128], F16)
            nc.sync.dma_start(out=w1[:], in_=W1[:])
            # H256 split into its two 128-row slabs (contraction halves)
            # two distinct assignments: pool slots are tagged by source
            # variable name, and a bufs=1 pool deadlocks if two live tiles
            # share a tag (constants are never released)
            hj0 = cpool.tile([128, 256], F16)
            nc.sync.dma_start(out=hj0[:], in_=HJ2[0:128, :])
            hj1 = cpool.tile([128, 256], F16)
            nc.sync.dma_start(out=hj1[:], in_=HJ2[128:256, :])
            hj = [hj0, hj1]
            xconst = None
            if mode == "compute":
                xconst = cpool.tile([128, FREE], F16)
                nc.sync.dma_start(
                    out=xconst[:].rearrange("p (a j) -> p a j",
                                            a=A_PER_GROUP, j=256),
                    in_=X_re[0],
                )

            def flush_b(state):
                """Emit the B-stage (4 accumulating MM-B + ACT copy +
                maybe store) for a previously A-staged 8-row slice."""
                if state is None:
                    return
                sa, yw_3d_, yw_, a_, g_ = state
                pb = psB.tile([128, 256], F32)
                for ch in range(2):
                    for h in range(2):
                        nc.tensor.matmul(
                            pb[:, ch * 128:(ch + 1) * 128],
                            lhsT=sa[:, h * 128:(h + 1) * 128],
                            rhs=hj[h][:, ch * 128:(ch + 1) * 128],
                            start=(h == 0),
                            stop=(h == 1),
                        )
                nc.scalar.copy(
                    out=yw_[:, a_ * 256:(a_ + 1) * 256], in_=pb[:]
                )
                if a_ == A_PER_GROUP - 1 and mode != "compute":
                    # stores ride the ACT HWDGE ring; loads own the SP ring
                    nc.scalar.dma_start(out=Y_re[g_], in_=yw_3d_)

            def emit_body():
              # 1-stage software pipeline: each slice's MM-B block is
              # emitted after the NEXT slice's MM-A block, so the PE FIFO
              # never stalls on the DVE PSUM->SBUF copy in between.
              prev = None
              for g in range(GROUPS):
                if mode == "compute":
                    xw = xconst
                else:
                    xw = xpool.tile([128, FREE], F16)
                xw_3d = xw[:].rearrange("p (a j) -> p a j", a=A_PER_GROUP, j=256)
                if mode != "compute":
                    nc.sync.dma_start(out=xw_3d, in_=X_re[g])
                yw = ypool.tile([128, FREE], F16)
                yw_3d = yw[:].rearrange("p (a j) -> p a j", a=A_PER_GROUP, j=256)
                if mode == "dma":
                    nc.scalar.dma_start(out=Y_re[g], in_=xw_3d)
                    continue
                for a in range(A_PER_GROUP):
                    # stage A: the two jc-halves of this slice, one MM each,
                    # into the two column halves of one psum tile
                    pa = psA.tile([128, 256], F32)
                    for h in range(2):
                        nc.tensor.matmul(
                            pa[:, h * 128:(h + 1) * 128],
                            lhsT=xw[:, a * 256 + h * 128:a * 256 + (h + 1) * 128],
                            rhs=w1[:],
                            start=True,
                            stop=True,
                        )
                    flush_b(prev)
                    sa = spool.tile([128, 256], F16)
                    nc.vector.tensor_copy(out=sa[:], in_=pa[:])
                    prev = (sa, yw_3d, yw, a, g)
              flush_b(prev)

            if loop_reps is None:
                emit_body()
            else:
                with tc.For_i(0, loop_reps, 1):
                    emit_body()

    nc.compile()
    return nc


_NC = None


def _get_nc():
    global _NC
    if _NC is None:
        _NC = _build_bass()
    return _NC


def make_in_maps(X: np.ndarray) -> list[dict]:
    """Shard X row-wise into 8 fp16 per-core input maps."""
    X = np.asarray(X, dtype=np.float32)
    assert X.shape == (ROWS, N), X.shape
    X16 = X.astype(np.float16)
    W1, HJ2 = _constants()
    return [
        {
            "X": X16[c * ROWS_PER_CORE:(c + 1) * ROWS_PER_CORE],
            "W1": W1,
            "HJ2": HJ2,
        }
        for c in range(N_CORES)
    ]


def run(X: np.ndarray, trace: bool = False):
    """Run the SPMD kernel on 8 cores; returns (Y, BassKernelResults)."""
    in_maps = make_in_maps(X)
    nc = _get_nc()
    res = run_bass_kernel_spmd(
        nc, in_maps, list(range(N_CORES)), trace=trace
    )
    Y = np.concatenate(
        [res.results[c]["Y"] for c in range(N_CORES)], axis=0
    ).astype(np.float32)
    return Y, res


def kernel(X, H=None, **_unused) -> np.ndarray:
    """Full-input entry point: X (8192, 4096) f32, H ignored (H is the
    deterministic normalized Hadamard matrix, synthesized on device)."""
    Y, _ = run(X, trace=False)
    return Y


# revision 24
# speedup vs baseline: 1.4790x; 1.0157x over previous
"""Trainium2 Bass kernel for nn_HadamardTransform: Y = X @ H4096_normalized.

Algorithm: H4096 (Sylvester, normalized) factors exactly as the Kronecker
product H16n (x) H256n.  Each row x of X, reshaped row-major to R[16, 256],
transforms as  Y_mat = G16 @ R @ H256u  with G16 = 2^-6 * H16u (all of the
2^-6 normalization folded into the 16-side so H256u stays exactly +-1).

On-chip scheme per 8-row slice (partition p = 16*b + ip, b in [8] rows,
ip in [16] column superblocks of 256; free = (a, jc in [256])):
  MM-A (x2, one per 128-half jh of jc):
      psumA[jh-half] = T_jh.T @ W1      (W1 = I8 (x) G16, block-diagonal)
      -> psumA[jc, (b,i')] : the 16-transform, emerging jc-on-partitions
      (the two jc halves land in different psum tiles' partition meaning,
       packed as columns [0:128]/[128:256] of one psum bank)
  MM-B (x4, accumulating pairs): psumB[:, ch] = sum_h sah.T @ H256[h, ch]
      -> psumB[(b,i'), jc'] : the 256-transform, natural output layout
No transposes are needed anywhere; the fixed matrices are the moving
operands, the per-slice data is the stationary operand.

Why 16x256 rather than 32x128: the DMA descriptor economy.  Both layouts
move the same bytes, but contiguous runs here are 512 B (256 fp16) vs
256 B, halving descriptor count; measured on HW the 8-core DMA floor
drops from ~67 us to ~60.5 us (payload-bound at ~278 GB/s/core), at the
cost of a third PE pass (the 256-contraction needs two accumulating
matmuls).  PE and DMA both land near 60 us and overlap.

Everything on device is fp16: X is cast host-side (rel err ~5e-4 vs the
2e-2 gate), halving DMA traffic (the roofline here); fp16 moving
operands stream at 1 cycle/column through the PE vs 4 for fp32.  PSUM
accumulation stays fp32; the PSUM->SBUF evacuations cast to fp16 (DVE
for stage A, ACT for stage B).  Loads ride the SP HWDGE ring, stores the
ACT HWDGE ring.

Sharding: X's 8192 rows split into 8 contiguous shards of 1024 rows, one
per NeuronCore (pure data parallelism, no collectives).
"""

import sys

import numpy as np

try:
    import concourse.bass as bass
except ImportError:  # repo not on sys.path in a fresh grading dir
    sys.path.insert(0, "/opt/trn_rl_repo")
    import concourse.bass as bass

import concourse.mybir as mybir
import concourse.tile as tile
from concourse import bacc
from concourse.bass_utils import run_bass_kernel_spmd

N_CORES = 8
ROWS = 8192
N = 4096
ROWS_PER_CORE = ROWS // N_CORES  # 1024
ROWS_PER_GROUP = 64               # rows moved per DMA (512 KiB fp16)
A_PER_GROUP = ROWS_PER_GROUP // 8  # 8-row slices per group
GROUPS = ROWS_PER_CORE // ROWS_PER_GROUP  # 16
F16 = mybir.dt.float16
F32 = mybir.dt.float32


def _hadamard_u(n: int) -> np.ndarray:
    """Unnormalized Sylvester Hadamard matrix (+-1 entries)."""
    H = np.array([[1.0]], dtype=np.float64)
    while H.shape[0] < n:
        H = np.block([[H, H], [H, -H]])
    return H


def _constants() -> tuple[np.ndarray, np.ndarray]:
    G16 = (2.0 ** -6) * _hadamard_u(16)         # fold full 2^-6 norm here
    W1 = np.kron(np.eye(8), G16).astype(np.float16)  # [128,128] block-diag
    HJ2 = _hadamard_u(256).astype(np.float16)        # [256,256] exact +-1
    return W1, HJ2


def _build_bass(loop_reps: int | None = None, mode: str = "full"):
    """loop_reps: if set, wrap the whole body in a HW For_i loop that
    repeats it loop_reps times (timing harness only — result unchanged
    since the same X is re-read).
    mode: "full" (real kernel), "dma" (loads+stores only, no compute),
    "compute" (matmuls+copies on resident tiles, no X/Y DMA)."""
    nc = bacc.Bacc("TRN2", target_bir_lowering=False, debug=False)

    X = nc.dram_tensor("X", [ROWS_PER_CORE, N], F16, kind="ExternalInput")
    W1 = nc.dram_tensor("W1", [128, 128], F16, kind="ExternalInput")
    HJ2 = nc.dram_tensor("HJ2", [256, 256], F16, kind="ExternalInput")
    Y = nc.dram_tensor("Y", [ROWS_PER_CORE, N], F16, kind="ExternalOutput")

    # row r = 64*g + 8*a + b ; column c = 256*ip + jc
    # SBUF group tile: partition p = 16*b + ip, free f = 256*a + jc
    X_re = X[:].rearrange(
        "(g a b) (i j) -> g b i a j", a=A_PER_GROUP, b=8, i=16, j=256
    )
    Y_re = Y[:].rearrange(
        "(g a b) (i j) -> g b i a j", a=A_PER_GROUP, b=8, i=16, j=256
    )
    FREE = A_PER_GROUP * 256  # free size of a group tile

    with tile.TileContext(nc) as tc:
        with (
            tc.tile_pool(name="consts", bufs=1) as cpool,
            tc.tile_pool(name="xin", bufs=5) as xpool,
            tc.tile_pool(name="yout", bufs=4) as ypool,
            tc.tile_pool(name="mid", bufs=6) as spool,
            tc.tile_pool(name="psA", bufs=3, space="PSUM") as psA,
            tc.tile_pool(name="psB", bufs=3, space="PSUM") as psB,
        ):
            w1 = cpool.tile([128, 128], F16)
            nc.sync.dma_start(out=w1[:], in_=W1[:])
            # H256 split into its two 128-row slabs (contraction halves)
            # two distinct assignments: pool slots are tagged by source
            # variable name, and a bufs=1 pool deadlocks if two live tiles
            # share a tag (constants are never released)
            hj0 = cpool.tile([128, 256], F16)
            nc.sync.dma_start(out=hj0[:], in_=HJ2[0:128, :])
            hj1 = cpool.tile([128, 256], F16)
            nc.sync.dma_start(out=hj1[:], in_=HJ2[128:256, :])
            hj = [hj0, hj1]
            xconst = None
            if mode == "compute":
                xconst = cpool.tile([128, FREE], F16)
                nc.sync.dma_start(
                    out=xconst[:].rearrange("p (a j) -> p a j",
                                            a=A_PER_GROUP, j=256),
                    in_=X_re[0],
                )

            def flush_b(state):
                """Emit the B-stage (4 accumulating MM-B + ACT copy +
                maybe store) for a previously A-staged 8-row slice."""
                if state is None:
                    return
                sa, yw_3d_, yw_, u_, g_ = state
                pb = psB.tile([128, 512], F32)
                for s in range(2):
                    for ch in range(2):
                        for h in range(2):
                            nc.tensor.matmul(
                                pb[:, (2 * s + ch) * 128:(2 * s + ch + 1) * 128],
                                lhsT=sa[:, (2 * s + h) * 128:(2 * s + h + 1) * 128],
                                rhs=hj[h][:, ch * 128:(ch + 1) * 128],
                                start=(h == 0),
                                stop=(h == 1),
                            )
                nc.scalar.copy(
                    out=yw_[:, u_ * 512:(u_ + 1) * 512], in_=pb[:]
                )
                if u_ == A_PER_GROUP // 2 - 1 and mode != "compute":
                    # stores ride the ACT HWDGE ring; loads own the SP ring
                    nc.scalar.dma_start(out=Y_re[g_], in_=yw_3d_)

            def emit_body():
              # 1-stage software pipeline: each slice's MM-B block is
              # emitted after the NEXT slice's MM-A block, so the PE FIFO
              # never stalls on the DVE PSUM->SBUF copy in between.
              prev = None
              for g in range(GROUPS):
                if mode == "compute":
                    xw = xconst
                else:
                    xw = xpool.tile([128, FREE], F16)
                xw_3d = xw[:].rearrange("p (a j) -> p a j", a=A_PER_GROUP, j=256)
                if mode != "compute":
                    nc.sync.dma_start(out=xw_3d, in_=X_re[g])
                yw = ypool.tile([128, FREE], F16)
                yw_3d = yw[:].rearrange("p (a j) -> p a j", a=A_PER_GROUP, j=256)
                if mode == "dma":
                    nc.scalar.dma_start(out=Y_re[g], in_=xw_3d)
                    continue
                for u in range(A_PER_GROUP // 2):
                    # stage A for a PAIR of 8-row slices (full psum bank,
                    # one 512-wide DVE evacuation per 16 rows)
                    pa = psA.tile([128, 512], F32)
                    for s in range(2):
                        a = 2 * u + s
                        for h in range(2):
                            nc.tensor.matmul(
                                pa[:, (2 * s + h) * 128:(2 * s + h + 1) * 128],
                                lhsT=xw[:, a * 256 + h * 128:a * 256 + (h + 1) * 128],
                                rhs=w1[:],
                                start=True,
                                stop=True,
                            )
                    flush_b(prev)
                    sa = spool.tile([128, 512], F16)
                    nc.vector.tensor_copy(out=sa[:], in_=pa[:])
                    prev = (sa, yw_3d, yw, u, g)
              flush_b(prev)

            if loop_reps is None:
                emit_body()
            else:
                with tc.For_i(0, loop_reps, 1):
                    emit_body()

    nc.compile()
    return nc


_NC = None


def _get_nc():
    global _NC
    if _NC is None:
        _NC = _build_bass()
    return _NC


def make_in_maps(X: np.ndarray) -> list[dict]:
    """Shard X row-wise into 8 fp16 per-core input maps."""
    X = np.asarray(X, dtype=np.float32)
    assert X.shape == (ROWS, N), X.shape
    X16 = X.astype(np.float16)
    W1, HJ2 = _constants()
    return [
        {
            "X": X16[c * ROWS_PER_CORE:(c + 1) * ROWS_PER_CORE],
            "W1": W1,
            "HJ2": HJ2,
        }
        for c in range(N_CORES)
    ]


def run(X: np.ndarray, trace: bool = False):
    """Run the SPMD kernel on 8 cores; returns (Y, BassKernelResults)."""
    in_maps = make_in_maps(X)
    nc = _get_nc()
    res = run_bass_kernel_spmd(
        nc, in_maps, list(range(N_CORES)), trace=trace
    )
    Y = np.concatenate(
        [res.results[c]["Y"] for c in range(N_CORES)], axis=0
    ).astype(np.float32)
    return Y, res


def kernel(X, H=None, **_unused) -> np.ndarray:
    """Full-input entry point: X (8192, 4096) f32, H ignored (H is the
    deterministic normalized Hadamard matrix, synthesized on device)."""
    Y, _ = run(X, trace=False)
    return Y
